# revision 24
# baseline (speedup 1.0000x reference)
"""Block-sparse MoE (top-2 of 8 experts, SwiGLU FFN) for 8 Trainium2 NeuronCores.

Strategy: expert-parallel. Core e owns expert e (its w13/w2 shards). Every core:
  1. computes router logits/softmax/top-2 fully on device (fp32 matmuls, exact
     enough for stable top-k picks),
  2. compacts the token ids + combine weights for ITS expert on device:
     a triangular-ones matmul gives each selected token its compact slot,
     then per (token-tile, slot-tile) selection-indicator matmuls
     (ind[t, j] = [slot[t] == j]) compact token ids, combine weights and slot
     occupancy in one accumulation — no DRAM scatter/gather roundtrip,
  3. indirect-DMA gathers the selected token rows of x, PE-transposes them,
  4. runs the SwiGLU FFN on the <=CAP gathered tokens (float16 matmuls,
     fp32 PSUM accumulation),
  5. scales rows by the renormalized top-2 combine weight and writes the
     compact [CAP, H] result + its token-index list as outputs.
Host side reshapes/shards inputs and scatter-adds the 8 compact outputs
(row j of core e goes to token idx[j]; idx >= T marks an empty slot).

Per-core expert identity is data-driven (the gate matrix columns are permuted
so each core's own expert is column 0), so a single SPMD program runs on all 8
cores.
"""

from contextlib import ExitStack

import numpy as np

import concourse.bass as bass
import concourse.tile as tile
from concourse import bacc, mybir
from concourse.bass import IndirectOffsetOnAxis
from concourse.masks import make_identity

P = 128
FULL = dict(T=2048, H=1024, F=3584, E=8, CAP=640, NCHUNK=320)
SMALL = dict(T=256, H=256, F=512, E=8, CAP=128, NCHUNK=64)

F32 = mybir.dt.float32
F32R = mybir.dt.float32r
F16 = mybir.dt.float16
I32 = mybir.dt.int32
AX = mybir.AxisListType
OP = mybir.AluOpType
ACT = mybir.ActivationFunctionType


def build_nc(cfg, mm_dtype=F16, num_devices=8):
    T, H, F, E, CAP = cfg["T"], cfg["H"], cfg["F"], cfg["E"], cfg["CAP"]
    NCH = cfg["NCHUNK"]
    NT, KH, NF, NCT = T // P, H // P, F // P, CAP // P
    DUMP = CAP  # "not routed here" slot value before clamping
    TPAD = T  # zero row of xpad; empty slots gather it
    MMD = mm_dtype

    nc = bacc.Bacc("TRN2", target_bir_lowering=False, debug=False,
                   num_devices=num_devices)

    xT = nc.dram_tensor("xT", [H, T], F32, kind="ExternalInput").ap()
    xpad = nc.dram_tensor("xpad", [T + 1, H], F32, kind="ExternalInput").ap()
    gwT = nc.dram_tensor("gwT", [H, E], F32, kind="ExternalInput").ap()
    w13p = nc.dram_tensor("w13p", [NF, P, 2 * H], MMD,
                          kind="ExternalInput").ap()
    w2p = nc.dram_tensor("w2p", [NF, P, H], MMD, kind="ExternalInput").ap()
    out_c = nc.dram_tensor("out_c", [CAP, H], F32, kind="ExternalOutput").ap()
    idx_o = nc.dram_tensor("idx_o", [CAP, 1], I32, kind="ExternalOutput").ap()

    tri_np = np.triu(np.ones((P, P), np.float32))  # tri[k, m] = 1 if k <= m
    tri_d = nc.inline_tensor(tri_np, name="tri").ap()

    # alternate direct DMAs across the two HWDGE queues (SP + Activation)
    dma_engs = [nc.sync, nc.scalar]

    with tile.TileContext(nc) as tc, ExitStack() as ctx:
        const = ctx.enter_context(tc.tile_pool(name="const", bufs=1))
        ident = const.tile([P, P], F32)
        make_identity(nc, ident[:])
        tri = const.tile([P, P], F32)
        nc.scalar.dma_start(tri[:], tri_d[:])

        persist = ctx.enter_context(tc.tile_pool(name="persist", bufs=1))
        rt = persist

        # ---------------- Phase R: router ----------------
        with tc.tile_pool(name="xt", bufs=1) as xt_pool, \
             tc.tile_pool(name="rpsum", bufs=2, space="PSUM") as rpsum, \
             tc.tile_pool(name="rsb", bufs=2) as rsb:
            gw = rsb.tile([P, KH, E], F32, tag="gw", bufs=1)
            nc.scalar.dma_start(gw[:], gwT.rearrange("(k p) e -> p k e", p=P))
            xts = []
            for k in range(KH):
                t = xt_pool.tile([P, T], F32, tag=f"xt{k}", name=f"xt{k}")
                dma_engs[k % 2].dma_start(t[:], xT[k * P:(k + 1) * P, :])
                xts.append(t)

            # gw stationary (8-col LDWEIGHTS is ~free); x streams as the
            # moving operand; logits come out expert-major and are
            # PE-transposed back to token-major tiles.
            RCH = min(512, T)
            NRC = T // RCH
            la = persist.tile([P, NT, E], F32, tag="logits")
            lgt_ps = []
            for ch in range(NRC):
                lp = rpsum.tile([E, RCH], F32, tag=f"lgt{ch}",
                                name=f"lgt{ch}", bufs=1)
                lgt_ps.append(lp)
            for k in range(KH):
                for ch in range(NRC):
                    nc.tensor.matmul(
                        lgt_ps[ch][:], lhsT=gw[:, k, :],
                        rhs=xts[k][:, ch * RCH:(ch + 1) * RCH],
                        start=(k == 0), stop=(k == KH - 1))
            lgt_sb = rsb.tile([E, T], F32, tag="lgt_sb", bufs=1)
            for ch in range(NRC):
                nc.vector.tensor_copy(lgt_sb[:, ch * RCH:(ch + 1) * RCH],
                                      lgt_ps[ch][:])
            for i in range(NT):
                lg = rpsum.tile([P, E], F32, tag="lg")
                nc.tensor.transpose(lg[:], lgt_sb[:, i * P:(i + 1) * P],
                                    ident[:E, :E])
                nc.vector.tensor_copy(la[:, i, :], lg[:])

        ffn = ctx.enter_context(tc.tile_pool(name="ffn", bufs=1))

        # batched softmax / top-2 / combine over [P, NT, E]
        m1 = rt.tile([P, NT, 1], F32, tag="m1")
        nc.vector.tensor_reduce(m1[:], la[:], axis=AX.X, op=OP.max)
        zc = rt.tile([P, NT, E], F32, tag="zc")
        nc.vector.tensor_tensor(zc[:], la[:], m1[:].to_broadcast([P, NT, E]),
                                op=OP.subtract)
        mask1 = rt.tile([P, NT, E], F32, tag="mask1")
        nc.vector.tensor_scalar(mask1[:], zc[:], 0.0, None, op0=OP.is_ge)
        low = rt.tile([P, NT, E], F32, tag="low")
        nc.vector.tensor_scalar(low[:], mask1[:], -1e30, None, op0=OP.mult)
        nc.vector.tensor_tensor(low[:], zc[:], low[:], op=OP.add)
        m2 = rt.tile([P, NT, 1], F32, tag="m2")
        nc.vector.tensor_reduce(m2[:], low[:], axis=AX.X, op=OP.max)
        mask = rt.tile([P, NT, E], F32, tag="mask")
        nc.vector.tensor_tensor(mask[:], zc[:], m2[:].to_broadcast([P, NT, E]),
                                op=OP.is_ge)
        ez = rt.tile([P, NT, E], F32, tag="ez")
        nc.scalar.activation(ez[:], zc[:], ACT.Exp)
        pm = rt.tile([P, NT, E], F32, tag="pm")
        nc.vector.tensor_tensor(pm[:], ez[:], mask[:], op=OP.mult)
        s = rt.tile([P, NT, 1], F32, tag="s")
        nc.vector.tensor_reduce(s[:], pm[:], axis=AX.X, op=OP.add)
        r = rt.tile([P, NT, 1], F32, tag="r")
        nc.vector.reciprocal(r[:], s[:])
        # one Newton step: r = r * (2 - s * r)
        nwt = rt.tile([P, NT, 1], F32, tag="nwt")
        nc.vector.tensor_tensor(nwt[:], s[:], r[:], op=OP.mult)
        nc.vector.tensor_scalar(nwt[:], nwt[:], -1.0, 2.0, op0=OP.mult,
                                op1=OP.add)
        nc.vector.tensor_tensor(r[:], r[:], nwt[:], op=OP.mult)

        c_cols = rt.tile([P, NT, 1], F32, tag="c_cols")
        nc.vector.tensor_tensor(c_cols[:], pm[:, :, 0:1], r[:], op=OP.mult)
        m_col = rt.tile([P, NT], F32, tag="m_col")
        nc.vector.tensor_copy(m_col[:], mask[:, :, 0:1])

        # ---------------- Phase D: compact slots / ids / weights ------------
        idx_sb, cg = [], []
        with tc.tile_pool(name="dpsum", bufs=1, space="PSUM") as dpsum, \
             tc.tile_pool(name="apsum", bufs=1, space="PSUM") as apsum, \
             tc.tile_pool(name="dsb", bufs=2) as dsb:
            # global inclusive cumsum of the selection mask -> slot per token
            s_ps = dpsum.tile([P, NT], F32, tag="s_ps")
            nc.tensor.matmul(s_ps[:], lhsT=tri[:], rhs=m_col[:], start=True,
                             stop=True)
            s_sb = rt.tile([P, NT], F32, tag="s_sb")
            nc.vector.tensor_copy(s_sb[:], s_ps[:])
            tot_ps = dpsum.tile([1, NT], F32, tag="tot_ps")
            nc.tensor.matmul(tot_ps[:], lhsT=tri[:, P - 1:P], rhs=m_col[:],
                             start=True, stop=True)
            tot_sb = rt.tile([1, NT], F32, tag="tot_sb")
            nc.vector.tensor_copy(tot_sb[:], tot_ps[:])
            zrow = rt.tile([1, NT], F32, tag="zrow")
            nc.vector.memset(zrow[:], 0.0)
            ic = rt.tile([1, NT], F32, tag="ic")
            nc.vector.tensor_tensor_scan(ic[:], tot_sb[:], zrow[:],
                                         initial=0.0, op0=OP.add, op1=OP.add)
            ex = rt.tile([1, NT], F32, tag="ex")
            nc.vector.tensor_tensor(ex[:], ic[:], tot_sb[:], op=OP.subtract)
            exb_ps = dpsum.tile([P, NT], F32, tag="exb")
            nc.tensor.matmul(exb_ps[:], lhsT=tri[0:1, :], rhs=ex[:],
                             start=True, stop=True)
            pos = rt.tile([P, NT], F32, tag="pos")
            nc.vector.tensor_tensor(pos[:], s_sb[:], exb_ps[:], op=OP.add)

            slotf = rt.tile([P, NT], F32, tag="slotf")
            nc.vector.tensor_scalar(slotf[:], pos[:], float(-1 - DUMP), None,
                                    op0=OP.add)
            nc.vector.tensor_tensor(slotf[:], slotf[:], m_col[:], op=OP.mult)
            nc.vector.tensor_scalar(slotf[:], slotf[:], float(DUMP),
                                    float(DUMP), op0=OP.add, op1=OP.min)

            # rhs columns per token-tile: [token-id, combine-w, 1]
            # (f32r: token ids <= 2048 stay exact; combine rounds to ~tf32)
            rhs3 = rt.tile([P, NT, 3], F32R, tag="rhs3")
            toki = rt.tile([P, NT, 1], I32, tag="toki")
            nc.gpsimd.iota(toki[:], pattern=[[P, NT], [0, 1]], base=0,
                           channel_multiplier=1)
            nc.vector.tensor_copy(rhs3[:, :, 0:1], toki[:])
            nc.vector.tensor_copy(rhs3[:, :, 1:2], c_cols[:])
            ones1 = rt.tile([P, NT, 1], F32, tag="ones1")
            nc.vector.memset(ones1[:], 1.0)
            nc.vector.tensor_copy(rhs3[:, :, 2:3], ones1[:])

            # jall[p, j] = j for every partition
            jall_i = dsb.tile([P, CAP], I32, tag="jall_i")
            nc.gpsimd.iota(jall_i[:], pattern=[[1, CAP]], base=0,
                           channel_multiplier=0)
            jall = dsb.tile([P, CAP], F32, tag="jall")
            nc.vector.tensor_copy(jall[:], jall_i[:])

            # accT[3, slot] += rhs3[:, i, :].T @ ind(i); rhs3 stationary
            # (3-col LDWEIGHTS ~free), indicator streams as moving operand.
            dchunks = []
            n0 = 0
            while n0 < CAP:
                nsz = min(256, CAP - n0)
                dchunks.append((n0, nsz))
                n0 += nsz
            accT_ps = [apsum.tile([3, nsz], F32, tag=f"accT{ci}",
                                  name=f"accT{ci}")
                       for ci, (n0, nsz) in enumerate(dchunks)]
            accT_sb = rt.tile([3, CAP], F32, tag="accT_sb")
            done_ct = 0
            for ci, (n0, nsz) in enumerate(dchunks):
                for i in range(NT):
                    ind = dsb.tile([P, nsz], F32R, tag="ind", bufs=3)
                    nc.vector.tensor_tensor(
                        ind[:], slotf[:, i:i + 1].to_broadcast([P, nsz]),
                        jall[:, n0:n0 + nsz], op=OP.is_equal)
                    nc.tensor.matmul(accT_ps[ci][:], lhsT=rhs3[:, i, :],
                                     rhs=ind[:], start=(i == 0),
                                     stop=(i == NT - 1))
                nc.vector.tensor_copy(accT_sb[:, n0:n0 + nsz], accT_ps[ci][:])
                while (done_ct + 1) * P <= n0 + nsz:
                    ct = done_ct
                    tp3 = dpsum.tile([P, 3], F32, tag="tp3", bufs=2)
                    nc.tensor.transpose(tp3[:],
                                        accT_sb[:, ct * P:(ct + 1) * P],
                                        ident[:3, :3])
                    acc_sb = rt.tile([P, 3], F32, tag=f"accsb{ct}",
                                     name=f"accsb{ct}")
                    nc.vector.tensor_copy(acc_sb[:], tp3[:])
                    # idx = raw + (1 - occ) * TPAD ; empty -> zero row
                    idxf = rt.tile([P, 1], F32, tag=f"idxf{ct}",
                                   name=f"idxf{ct}")
                    nc.vector.tensor_scalar(idxf[:], acc_sb[:, 2:3],
                                            float(-TPAD), float(TPAD),
                                            op0=OP.mult, op1=OP.add)
                    nc.vector.tensor_tensor(idxf[:], idxf[:],
                                            acc_sb[:, 0:1], op=OP.add)
                    ii = rt.tile([P, 1], I32, tag=f"idx{ct}",
                                 name=f"idx{ct}")
                    nc.vector.tensor_copy(ii[:], idxf[:])
                    idx_sb.append(ii)
                    cg.append(acc_sb[:, 1:2])
                    dma_engs[ct % 2].dma_start(
                        idx_o[ct * P:(ct + 1) * P, :], ii[:])
                    done_ct += 1

        # ---------------- Phase G: gather selected tokens + transpose -------
        xgT = []
        for k in range(KH):
            xgT.append(ffn.tile([P, CAP], MMD, tag=f"xgT{k}",
                                    name=f"xgT{k}"))
        with tc.tile_pool(name="gat", bufs=5) as gat, \
             tc.tile_pool(name="tpsum", bufs=2, space="PSUM") as tpsum:
            for ct in range(NCT):
                xg = gat.tile([P, H], F32, tag="xg")
                nc.gpsimd.indirect_dma_start(
                    out=xg[:], out_offset=None, in_=xpad[:, :],
                    in_offset=IndirectOffsetOnAxis(ap=idx_sb[ct][:, 0:1],
                                                   axis=0))
                for k in range(KH):
                    tp = tpsum.tile([P, P], F32, tag="tp")
                    nc.tensor.transpose(tp[:], xg[:, k * P:(k + 1) * P],
                                        ident[:])
                    nc.vector.tensor_copy(xgT[k][:, ct * P:(ct + 1) * P],
                                          tp[:])

        # w2 cache: read each f-block once, keep both h-halves in SBUF
        w2c = [ffn.tile([P, H], MMD, tag=f"w2c{f}", name=f"w2c{f}")
               for f in range(NF)]
        for f in range(NF):
            dma_engs[f % 2].dma_start(w2c[f][:], w2p[f, :, :])

        # ---------------- Phase F1: h = x @ w13.T, a = silu(g) * u ----------
        aT = [ffn.tile([P, CAP], MMD, tag=f"aT{f}", name=f"aT{f}")
              for f in range(NF)]
        if CAP > 256:
            nchunks = [(0, 256), (256, CAP - 256)]
        else:
            nchunks = [(0, CAP)]
        assert all(nsz <= 512 for _, nsz in nchunks)
        with tc.tile_pool(name="w13s", bufs=8) as w13s, \
             tc.tile_pool(name="gups", bufs=6, space="PSUM") as gups, \
             tc.tile_pool(name="silu", bufs=6) as silu_p:
            for f in range(NF):
                wgu = w13s.tile([P, 2 * H], MMD, tag="wgu")
                dma_engs[f % 2].dma_start(wgu[:], w13p[f, :, :])
                wg = wgu[:, :H]
                wu = wgu[:, H:]
                for n0, nsz in nchunks:
                    g_ps = gups.tile([P, 512], F32, tag="gu")
                    u_ps = gups.tile([P, 512], F32, tag="gu")
                    for k in range(KH):
                        nc.tensor.matmul(
                            g_ps[:, :nsz], lhsT=wg[:, k * P:(k + 1) * P],
                            rhs=xgT[k][:, n0:n0 + nsz],
                            start=(k == 0), stop=(k == KH - 1))
                    for k in range(KH):
                        nc.tensor.matmul(
                            u_ps[:, :nsz], lhsT=wu[:, k * P:(k + 1) * P],
                            rhs=xgT[k][:, n0:n0 + nsz],
                            start=(k == 0), stop=(k == KH - 1))
                    sg = silu_p.tile([P, 512], F32, tag="sg")
                    nc.scalar.activation(sg[:, :nsz], g_ps[:, :nsz],
                                         ACT.Sigmoid)
                    nc.vector.tensor_tensor(sg[:, :nsz], sg[:, :nsz],
                                            g_ps[:, :nsz], op=OP.mult)
                    nc.vector.tensor_tensor(aT[f][:, n0:n0 + nsz],
                                            sg[:, :nsz], u_ps[:, :nsz],
                                            op=OP.mult)

        # ---------------- Phase F2: y = a @ w2.T, scale, write --------------
        HH = H // 2  # two h-half passes keep PSUM within 8 banks
        out_sb = [ffn.tile([P, H], F32, tag=f"osb{ct}", name=f"osb{ct}")
                  for ct in range(NCT)]
        with tc.tile_pool(name="ypsum", bufs=1, space="PSUM") as ypsum:
            for hh in range(2):
                y_ps = [ypsum.tile([P, HH], F32, tag=f"y{ct}", name=f"y{ct}")
                        for ct in range(NCT)]
                for f in range(NF):
                    for ct in range(NCT):
                        nc.tensor.matmul(
                            y_ps[ct][:], lhsT=aT[f][:, ct * P:(ct + 1) * P],
                            rhs=w2c[f][:, hh * HH:(hh + 1) * HH],
                            start=(f == 0), stop=(f == NF - 1))
                for ct in range(NCT):
                    nc.vector.tensor_scalar(
                        out_sb[ct][:, hh * HH:(hh + 1) * HH], y_ps[ct][:],
                        cg[ct][:, 0:1], None, op0=OP.mult)
                    dma_engs[ct % 2].dma_start(
                        out_c[ct * P:(ct + 1) * P, hh * HH:(hh + 1) * HH],
                        out_sb[ct][:, hh * HH:(hh + 1) * HH])

    nc.compile()
    return nc


def make_core_inputs(cfg, x, gate_w, w13, w2, core, mm_np=np.float16):
    T, H, F, E = cfg["T"], cfg["H"], cfg["F"], cfg["E"]
    NF = F // P
    NB = 2 * NF
    KH = H // P
    e = core
    perm = [e] + [i for i in range(E) if i != e]
    xT = np.ascontiguousarray(x.T)
    xpad = np.concatenate([x, np.zeros((1, H), np.float32)], axis=0)
    gwT = np.ascontiguousarray(gate_w[perm].T)
    w13b = (w13[e].reshape(NB, P, KH, P).transpose(0, 3, 2, 1)
            .reshape(NB, P, H).astype(mm_np, copy=False))
    w13p = np.ascontiguousarray(
        np.concatenate([w13b[:NF], w13b[NF:]], axis=2))
    w2p = np.ascontiguousarray(w2[e].T.reshape(NF, P, H).astype(mm_np,
                                                                copy=False))
    return {"xT": xT, "xpad": xpad, "gwT": gwT, "w13p": w13p, "w2p": w2p}


_NC_CACHE = {}


def run(x, gate_w, w13, w2, **spmd_kwargs):
    from concourse.bass_utils import run_bass_kernel_spmd

    cfg = FULL
    key = "full"
    if key not in _NC_CACHE:
        _NC_CACHE[key] = build_nc(cfg)
    nc = _NC_CACHE[key]
    x = np.ascontiguousarray(np.asarray(x, np.float32))
    gate_w = np.ascontiguousarray(np.asarray(gate_w, np.float32))
    w13 = np.ascontiguousarray(np.asarray(w13, np.float32))
    w2 = np.ascontiguousarray(np.asarray(w2, np.float32))
    in_maps = [make_core_inputs(cfg, x, gate_w, w13, w2, c) for c in range(8)]
    res = run_bass_kernel_spmd(nc, in_maps, core_ids=list(range(8)),
                               **spmd_kwargs)
    T = cfg["T"]
    acc = np.zeros((T, cfg["H"]), np.float32)
    for c in range(8):
        y = res.results[c]["out_c"]
        idx = res.results[c]["idx_o"][:, 0]
        m = idx < T
        acc[idx[m]] += y[m]
    return acc, res


def kernel(x, gate_w, w13, w2):
    acc, _ = run(x, gate_w, w13, w2)
    return acc
